# revision 1
# baseline (speedup 1.0000x reference)
"""Trainium2 Bass kernel for heterogeneous GNN (GAT + FFN) over 8 NeuronCores.

Strategy:
  - Relabel nodes so each core owns the nodes whose embedding rows fall in its
    row-shard of the embedding tables (tables row-sharded 8 ways).
  - Launch 1 (SPMD x8): gather embedding rows, project into hidden space with
    fused weights W'' = W_t @ [Wg | Wg@Al | Wg@Ar], emit H'[node] = [h|el|er]
    (144 f32) per node.
  - Host: concatenate per-core H' into the global node table (index work only).
  - Launch 2 (SPMD x8): edges with sentence dst only, sharded by dst owner,
    sorted by dst, padded to 128-edge slabs.  Gather H'[src] + er[dst], edge
    weights s = exp(leaky_relu(el+er)), segment softmax via one-hot mask
    matmuls accumulating [128-dst-window x (h.128 | den.8)] in PSUM, then
    normalize + ELU + FFN (residual) + logits, all in transposed layout.
"""

import os
import numpy as np

import concourse.bacc as bacc
import concourse.bass as bass
import concourse.mybir as mybir
import concourse.tile as tile


def _run_spmd(nc, in_maps, n_cores=8, bench=0):
    """Execute a compiled Bass program on n_cores via PJRT (axon).

    Mirrors bass2jax.run_bass_via_pjrt's multi-core path, but pre-places
    inputs on device so repeated timed calls measure NEFF execution only.
    Returns (results_per_core, best_exec_seconds or None).
    """
    import time as _time
    import jax
    from jax.sharding import Mesh, PartitionSpec, NamedSharding
    from jax.experimental.shard_map import shard_map
    from concourse import bass2jax as b2j
    from concourse import mybir as mb

    b2j.install_neuronx_cc_hook()
    part_name = nc.partition_id_tensor.name if nc.partition_id_tensor else None
    in_names, out_names, out_avals, zero_outs = [], [], [], []
    for alloc in nc.m.functions[0].allocations:
        if not isinstance(alloc, mb.MemoryLocationSet):
            continue
        name = alloc.memorylocations[0].name
        if alloc.kind == "ExternalInput":
            if name != part_name:
                in_names.append(name)
        elif alloc.kind == "ExternalOutput":
            out_names.append(name)
            shape = tuple(alloc.tensor_shape)
            dtype = mb.dt.np(alloc.dtype)
            out_avals.append(jax.core.ShapedArray(shape, dtype))
            zero_outs.append(np.zeros(shape, dtype))
    n_params = len(in_names)
    n_outs = len(out_avals)
    all_names = in_names + out_names
    if part_name is not None:
        all_names = all_names + [part_name]

    def _body(*args):
        operands = list(args)
        if part_name is not None:
            operands.append(b2j.partition_id_tensor())
        outs = b2j._bass_exec_p.bind(
            *operands,
            out_avals=tuple(out_avals),
            in_names=tuple(all_names),
            out_names=tuple(out_names),
            lowering_input_output_aliases=(),
            sim_require_finite=True,
            sim_require_nnan=True,
            nc=nc,
        )
        return tuple(outs)

    devices = jax.devices()[:n_cores]
    mesh = Mesh(np.asarray(devices), ("core",))
    donate = tuple(range(n_params, n_params + n_outs))
    sharded = jax.jit(
        shard_map(_body, mesh=mesh,
                  in_specs=(PartitionSpec("core"),) * (n_params + n_outs),
                  out_specs=(PartitionSpec("core"),) * n_outs,
                  check_rep=False),
        donate_argnums=donate, keep_unused=True)
    spec = NamedSharding(mesh, PartitionSpec("core"))
    concat_in = [
        jax.device_put(
            np.concatenate([np.asarray(in_maps[c][nm]) for c in range(n_cores)],
                           axis=0), spec)
        for nm in in_names
    ]
    def _zeros():
        return [jax.device_put(
                    np.zeros((n_cores * z.shape[0], *z.shape[1:]), z.dtype),
                    spec)
                for z in zero_outs]

    out_arrs = sharded(*concat_in, *_zeros())
    jax.block_until_ready(out_arrs)
    results = [
        {nm: np.asarray(out_arrs[i]).reshape(n_cores, *out_avals[i].shape)[c]
         for i, nm in enumerate(out_names)}
        for c in range(n_cores)
    ]
    best = None
    for _ in range(bench):
        zz = _zeros()
        jax.block_until_ready(zz)
        t0 = _time.perf_counter()
        oo = sharded(*concat_in, *zz)
        jax.block_until_ready(oo)
        dt_s = _time.perf_counter() - t0
        best = dt_s if best is None or dt_s < best else best
    return results, best

N_FEAT, N_SENT, N_USER, N_ITEM = 60000, 100000, 5000, 5000
N_NODES = N_FEAT + N_SENT + N_USER + N_ITEM
FEAT_NUM, SENT_NUM, USER_NUM, ITEM_NUM = 60000, 200000, 100000, 50000
HEADS, DH, HID = 8, 16, 128
FFN = 512
NCORES = 8
ROW = 144  # h(128) | el(8) | er(8)
F32 = mybir.dt.float32
F32R = mybir.dt.float32r
I32 = mybir.dt.int32

CHUNK = 16    # slabs per phase-2 chunk (2048 edges)
MACRO = 512   # nodes per phase-1 macro tile
NODE_ALIGN = 512

LAST_STATS = {}


def _ru(x, m):
    return ((x + m - 1) // m) * m


# ----------------------------------------------------------------------------
# launch 1: gather + project -> H'[node] = [h | el | er]
# ----------------------------------------------------------------------------

def _build_launch1(P_S, P_F, P_U, P_I, shard_rows, proj_dtype=F32R):
    nc = bacc.Bacc("TRN2", target_bir_lowering=False, debug=False,
                   enable_asserts=False)

    types = [
        ("s", P_S, 768, shard_rows["s"]),
        ("f", P_F, 300, shard_rows["f"]),
        ("u", P_U, 64, shard_rows["u"]),
        ("i", P_I, 64, shard_rows["i"]),
    ]
    core_rows = P_S + P_F + P_U + P_I

    tbl, idxd, wtT = {}, {}, {}
    for t, P_t, D_t, R_t in types:
        tbl[t] = nc.dram_tensor(f"tbl_{t}", [R_t, D_t], F32, kind="ExternalInput").ap()
        idxd[t] = nc.dram_tensor(f"idx_{t}", [128, P_t // 128], I32, kind="ExternalInput").ap()
        wtT[t] = nc.dram_tensor(f"wtT_{t}", [HID, D_t], F32, kind="ExternalInput").ap()
    wgT_d = nc.dram_tensor("wgT", [HID, HID], F32, kind="ExternalInput").ap()
    wg_d = nc.dram_tensor("wg", [HID, HID], F32, kind="ExternalInput").ap()
    alr_d = nc.dram_tensor("alr", [HID, 16], F32, kind="ExternalInput").ap()
    ident_d = nc.dram_tensor("ident", [128, 128], F32, kind="ExternalInput").ap()
    H_d = nc.dram_tensor("H", [core_rows, ROW], F32, kind="ExternalOutput").ap()

    with tile.TileContext(nc) as tc:
        with (
            tc.tile_pool(name="const", bufs=1) as cpool,
            tc.tile_pool(name="x", bufs=2) as xpool,
            tc.tile_pool(name="xt", bufs=3) as xtpool,
            tc.tile_pool(name="ht", bufs=2) as htpool,
            tc.tile_pool(name="hrow", bufs=3) as hrowpool,
            tc.tile_pool(name="ps_t", bufs=2, space="PSUM") as ps_t,
            tc.tile_pool(name="ps_h", bufs=2, space="PSUM") as ps_h,
            tc.tile_pool(name="ps_e", bufs=2, space="PSUM") as ps_e,
            tc.tile_pool(name="ps_o", bufs=2, space="PSUM") as ps_o,
        ):
            ident = cpool.tile([128, 128], F32)
            nc.sync.dma_start(ident[:], ident_d[:])

            # Wg144 = [Wg | Wg@Al | Wg@Ar]
            wg144 = cpool.tile([HID, ROW], F32)
            wgT_t = cpool.tile([HID, HID], F32)
            alr_t = cpool.tile([HID, 16], F32)
            nc.sync.dma_start(wg144[:, 0:HID], wg_d[:])
            nc.sync.dma_start(wgT_t[:], wgT_d[:])
            nc.sync.dma_start(alr_t[:], alr_d[:])
            ps_alr = ps_e.tile([128, 16], F32, tag="pse")
            nc.tensor.matmul(ps_alr[0:HID, :], wgT_t[:], alr_t[:],
                             start=True, stop=True)
            nc.vector.tensor_copy(wg144[:, HID:ROW], ps_alr[0:HID, :])

            # W''_t = W_t @ Wg144  (k-chunk tiles [kc<=128, 144])
            wpp = {}
            for t, P_t, D_t, R_t in types:
                wtT_s = cpool.tile([HID, D_t], F32, tag=f"wtT{t}")
                nc.sync.dma_start(wtT_s[:], wtT[t][:])
                wpp[t] = []
                for k0 in range(0, D_t, 128):
                    kn = min(128, D_t - k0)
                    pw = ps_e.tile([128, ROW], F32, tag="pse")
                    nc.tensor.matmul(pw[0:kn, :], wtT_s[:, k0:k0 + kn],
                                     wg144[:], start=True, stop=True)
                    wt = cpool.tile([128, ROW], F32R, tag=f"wpp{t}{k0}")
                    nc.vector.tensor_copy(wt[0:kn, :], pw[0:kn, :])
                    wpp[t].append((k0, kn, wt))

            row_off = {"s": 0, "f": P_S, "u": P_S + P_F, "i": P_S + P_F + P_U}
            idx_sb = {}
            for t, P_t, D_t, R_t in types:
                idx_sb[t] = cpool.tile([128, P_t // 128], I32, tag=f"idx{t}",
                                       name=f"idx_sb_{t}")
                nc.sync.dma_start(idx_sb[t][:], idxd[t][:])

            G = MACRO // 128
            for t, P_t, D_t, R_t in types:
                for m in range(P_t // MACRO):
                    x_t = xpool.tile([128, G * D_t], F32, tag="x")
                    for g in range(G):
                        nc.gpsimd.indirect_dma_start(
                            out=x_t[:, g * D_t:(g + 1) * D_t], out_offset=None,
                            in_=tbl[t][:],
                            in_offset=bass.IndirectOffsetOnAxis(
                                ap=idx_sb[t][:, m * G + g:m * G + g + 1],
                                axis=0))
                    psh = ps_h.tile([128, MACRO], F32, tag="psh")
                    pse = ps_e.tile([128, MACRO], F32, tag="pse")
                    nkc = len(wpp[t])
                    for ki, (k0, kn, wt) in enumerate(wpp[t]):
                        pst = ps_t.tile([128, MACRO], F32, tag="pst")
                        for g in range(G):
                            nc.tensor.transpose(
                                out=pst[0:kn, g * 128:(g + 1) * 128],
                                in_=x_t[:, g * D_t + k0: g * D_t + k0 + kn],
                                identity=ident[:])
                        xt_t = xtpool.tile([128, MACRO], F32R, tag="xt")
                        nc.vector.tensor_copy(xt_t[0:kn, :], pst[0:kn, :])
                        nc.tensor.matmul(
                            psh[:], wt[0:kn, 0:HID], xt_t[0:kn, :],
                            start=(ki == 0), stop=(ki == nkc - 1))
                        nc.tensor.matmul(
                            pse[0:16, :], wt[0:kn, HID:ROW], xt_t[0:kn, :],
                            start=(ki == 0), stop=(ki == nkc - 1))
                    ht_t = htpool.tile([128, MACRO], F32, tag="ht")
                    et_t = htpool.tile([16, MACRO], F32, tag="et")
                    nc.vector.tensor_copy(ht_t[:], psh[:])
                    nc.vector.tensor_copy(et_t[:], pse[0:16, :])
                    for g in range(G):
                        pso = ps_o.tile([128, ROW], F32, tag="pso")
                        nc.tensor.transpose(
                            out=pso[:, 0:HID],
                            in_=ht_t[:, g * 128:(g + 1) * 128],
                            identity=ident[:])
                        nc.tensor.transpose(
                            out=pso[:, HID:ROW],
                            in_=et_t[0:16, g * 128:(g + 1) * 128],
                            identity=ident[0:16, 0:16])
                        hrow = hrowpool.tile([128, ROW], F32, tag="hrow")
                        nc.vector.tensor_copy(hrow[:], pso[:])
                        r0 = row_off[t] + m * MACRO + g * 128
                        nc.sync.dma_start(H_d[r0:r0 + 128, :], hrow[:])
    nc.compile()
    return nc


# ----------------------------------------------------------------------------
# launch 2: edge aggregation + FFN + logits
# ----------------------------------------------------------------------------

def _build_launch2(grows, n_slabs, slab_win, P_S, agg_dtype=F32, ffn_dtype=F32R):
    nc = bacc.Bacc("TRN2", target_bir_lowering=False, debug=False,
                   enable_asserts=False)

    NW = P_S // 128
    H_d = nc.dram_tensor("H", [grows, ROW], F32, kind="ExternalInput").ap()
    src_d = nc.dram_tensor("src", [128, n_slabs], I32, kind="ExternalInput").ap()
    dst_d = nc.dram_tensor("dst", [128, n_slabs], I32, kind="ExternalInput").ap()
    rel_d = nc.dram_tensor("rel", [128, n_slabs], F32, kind="ExternalInput").ap()
    col_d = nc.dram_tensor("col", [128, 128], F32, kind="ExternalInput").ap()
    ident_d = nc.dram_tensor("ident", [128, 128], F32, kind="ExternalInput").ap()
    w1_d = nc.dram_tensor("w1", [HID, FFN], F32, kind="ExternalInput").ap()
    b1_d = nc.dram_tensor("b1c", [128, FFN // 128], F32, kind="ExternalInput").ap()
    w2_d = nc.dram_tensor("w2", [FFN, HID], F32, kind="ExternalInput").ap()
    b2_d = nc.dram_tensor("b2c", [128, 1], F32, kind="ExternalInput").ap()
    wh_d = nc.dram_tensor("wh", [HID, 1], F32, kind="ExternalInput").ap()
    whb_d = nc.dram_tensor("whb", [1, 1], F32, kind="ExternalInput").ap()
    lg_d = nc.dram_tensor("logits", [1, P_S], F32, kind="ExternalOutput").ap()

    first_slab, last_slab = {}, {}
    for s, w in enumerate(slab_win):
        first_slab.setdefault(w, s)
        last_slab[w] = s
    n_chunks = n_slabs // CHUNK

    with tile.TileContext(nc) as tc:
        with (
            tc.tile_pool(name="const", bufs=1) as cpool,
            tc.tile_pool(name="g", bufs=2) as gpool,
            tc.tile_pool(name="t", bufs=2) as tpool,
            tc.tile_pool(name="m", bufs=2) as mpool,
            tc.tile_pool(name="sm", bufs=3) as smpool,
            tc.tile_pool(name="ev", bufs=3) as evpool,
            tc.tile_pool(name="xf", bufs=2) as xfpool,
            tc.tile_pool(name="y", bufs=2) as ypool,
            tc.tile_pool(name="ps_w", bufs=2, space="PSUM") as ps_w,
            tc.tile_pool(name="ps_tp", bufs=1, space="PSUM") as ps_tp,
            tc.tile_pool(name="ps_y", bufs=2, space="PSUM") as ps_y,
            tc.tile_pool(name="ps_z", bufs=1, space="PSUM") as ps_z,
            tc.tile_pool(name="ps_l", bufs=1, space="PSUM") as ps_l,
        ):
            col = cpool.tile([128, 128], F32)
            ident = cpool.tile([128, 128], F32)
            w1_t = cpool.tile([HID, FFN], F32R)
            b1_t = cpool.tile([128, FFN // 128], F32)
            w2_t = cpool.tile([128, FFN], F32R)  # block j = W2[j*128:(j+1)*128,:]
            b2_t = cpool.tile([128, 1], F32)
            wh_t = cpool.tile([HID, 1], F32)
            whb_t = cpool.tile([1, 1], F32)
            src_sb = cpool.tile([128, n_slabs], I32)
            dst_sb = cpool.tile([128, n_slabs], I32)
            rel_sb = cpool.tile([128, n_slabs], F32)
            nc.sync.dma_start(col[:], col_d[:])
            nc.sync.dma_start(ident[:], ident_d[:])
            nc.gpsimd.dma_start(w1_t[:], w1_d[:])
            nc.sync.dma_start(b1_t[:], b1_d[:])
            for j in range(FFN // 128):
                nc.gpsimd.dma_start(w2_t[:, j * 128:(j + 1) * 128],
                                    w2_d[j * 128:(j + 1) * 128, :])
            nc.sync.dma_start(b2_t[:], b2_d[:])
            nc.sync.dma_start(wh_t[:], wh_d[:])
            nc.sync.dma_start(whb_t[:], whb_d[:])
            nc.sync.dma_start(src_sb[:], src_d[:])
            nc.sync.dma_start(dst_sb[:], dst_d[:])
            nc.sync.dma_start(rel_sb[:], rel_d[:])

            win_psum = {}
            xf_state = {"tile": None, "count": 0, "base": 0}

            def flush_ffn():
                nbat = xf_state["count"]
                if nbat == 0:
                    return
                xf = xf_state["tile"]
                nb = nbat * 128
                xfr = ypool.tile([128, 512], F32R, tag="xfr")
                nc.vector.tensor_copy(xfr[:, 0:nb], xf[:, 0:nb])
                yts = []
                for j in range(FFN // 128):
                    psy = ps_y.tile([128, 512], F32, tag="psy")
                    nc.tensor.matmul(
                        psy[:, 0:nb],
                        w1_t[:, j * 128:(j + 1) * 128],
                        xfr[:, 0:nb],
                        start=True, stop=True)
                    y_t = ypool.tile([128, 512], F32R, tag="y")
                    nc.scalar.activation(y_t[:, 0:nb], psy[:, 0:nb],
                                         mybir.ActivationFunctionType.Relu,
                                         bias=b1_t[:, j:j + 1])
                    yts.append(y_t)
                psz = ps_z.tile([128, 512], F32, tag="psz")
                for j in range(FFN // 128):
                    nc.tensor.matmul(
                        psz[:, 0:nb],
                        w2_t[:, j * 128:(j + 1) * 128],
                        yts[j][:, 0:nb],
                        start=(j == 0), stop=(j == FFN // 128 - 1))
                z_t = evpool.tile([128, 512], F32, tag="z")
                nc.scalar.activation(z_t[:, 0:nb], psz[:, 0:nb],
                                     mybir.ActivationFunctionType.Identity,
                                     bias=b2_t[:, 0:1])
                nc.vector.tensor_tensor(out=z_t[:, 0:nb], in0=z_t[:, 0:nb],
                                        in1=xf[:, 0:nb], op=mybir.AluOpType.add)
                psl = ps_l.tile([1, 512], F32, tag="psl")
                nc.tensor.matmul(psl[0:1, 0:nb], wh_t[:], z_t[:, 0:nb],
                                 start=True, stop=True)
                lg_t = evpool.tile([1, 512], F32, tag="lg")
                nc.scalar.activation(lg_t[0:1, 0:nb], psl[0:1, 0:nb],
                                     mybir.ActivationFunctionType.Identity,
                                     bias=whb_t[0:1, 0:1])
                b0 = xf_state["base"] * 128
                nc.sync.dma_start(lg_d[0:1, b0:b0 + nb], lg_t[0:1, 0:nb])
                xf_state["tile"] = None
                xf_state["count"] = 0

            def evacuate(w):
                psw = win_psum.pop(w)
                den = evpool.tile([128, 8], F32, tag="den")
                nc.vector.tensor_scalar(out=den[:], in0=psw[:, HID:HID + 8],
                                        scalar1=1e-9, scalar2=None,
                                        op0=mybir.AluOpType.add)
                rcp = evpool.tile([128, 8], F32, tag="rcp")
                nc.vector.reciprocal(rcp[:], den[:])
                ot = evpool.tile([128, 128], F32, tag="ot")
                nc.vector.tensor_tensor(
                    out=ot[:].rearrange("p (h r) -> p h r", h=8),
                    in0=psw[:, 0:HID].rearrange("p (h r) -> p h r", h=8),
                    in1=rcp[:].unsqueeze(2).broadcast_to([128, 8, 16]),
                    op=mybir.AluOpType.mult)
                neg = evpool.tile([128, 128], F32, tag="neg")
                nc.vector.tensor_scalar(out=neg[:], in0=ot[:], scalar1=0.0,
                                        scalar2=None, op0=mybir.AluOpType.min)
                emn = evpool.tile([128, 128], F32, tag="emn")
                nc.scalar.activation(emn[:], neg[:],
                                     mybir.ActivationFunctionType.Exp)
                pos = evpool.tile([128, 128], F32, tag="pos")
                nc.vector.tensor_scalar(out=pos[:], in0=ot[:], scalar1=0.0,
                                        scalar2=None, op0=mybir.AluOpType.max)
                nc.vector.tensor_scalar(out=emn[:], in0=emn[:], scalar1=-1.0,
                                        scalar2=None, op0=mybir.AluOpType.add)
                elu = evpool.tile([128, 128], F32, tag="elu")
                nc.vector.tensor_tensor(out=elu[:], in0=pos[:], in1=emn[:],
                                        op=mybir.AluOpType.add)
                pst = ps_tp.tile([128, 128], F32, tag="pstp")
                nc.tensor.transpose(out=pst[:], in_=elu[:], identity=ident[:])
                if xf_state["tile"] is None:
                    xf_state["tile"] = xfpool.tile([128, 512], F32, tag="xf",
                                                   name=f"xf_{w}")
                    xf_state["base"] = w
                k = xf_state["count"]
                nc.vector.tensor_copy(
                    xf_state["tile"][:, k * 128:(k + 1) * 128], pst[:])
                xf_state["count"] = k + 1
                if xf_state["count"] == 4:
                    flush_ffn()

            for c in range(n_chunks):
                g_t = gpool.tile([128, CHUNK * ROW], F32, tag="g")
                r_t = smpool.tile([128, CHUNK * 8], F32, tag="r")
                for g in range(CHUNK):
                    sc = c * CHUNK + g
                    nc.gpsimd.indirect_dma_start(
                        out=g_t[:, g * ROW:(g + 1) * ROW], out_offset=None,
                        in_=H_d[:],
                        in_offset=bass.IndirectOffsetOnAxis(
                            ap=src_sb[:, sc:sc + 1], axis=0))
                    nc.gpsimd.indirect_dma_start(
                        out=r_t[:, g * 8:(g + 1) * 8], out_offset=None,
                        in_=H_d[:],
                        in_offset=bass.IndirectOffsetOnAxis(
                            ap=dst_sb[:, sc:sc + 1], axis=0),
                        element_offset=HID + 8)
                gv = g_t[:].rearrange("p (g r) -> p g r", g=CHUNK)
                z_t = smpool.tile([128, CHUNK * 8], F32, tag="z8")
                nc.vector.tensor_tensor(
                    out=z_t[:].rearrange("p (g h) -> p g h", g=CHUNK),
                    in0=gv[:, :, HID:HID + 8],
                    in1=r_t[:].rearrange("p (g h) -> p g h", g=CHUNK),
                    op=mybir.AluOpType.add)
                zz_t = smpool.tile([128, CHUNK * 8], F32, tag="zz8")
                nc.scalar.mul(zz_t[:], z_t[:], 0.2)
                nc.vector.tensor_tensor(out=z_t[:], in0=z_t[:], in1=zz_t[:],
                                        op=mybir.AluOpType.max)
                s_t = smpool.tile([128, CHUNK * 8], F32, tag="s8")
                nc.scalar.activation(s_t[:], z_t[:],
                                     mybir.ActivationFunctionType.Exp)
                sv = s_t[:].rearrange("p (g h) -> p g h", g=CHUNK)
                t_t = tpool.tile([128, CHUNK * 136], F32, tag="t")
                tv = t_t[:].rearrange("p (g c) -> p g c", g=CHUNK)
                nc.vector.tensor_tensor(
                    out=tv[:, :, 0:HID].rearrange("p g (h r) -> p g h r", h=8),
                    in0=gv[:, :, 0:HID].rearrange("p g (h r) -> p g h r", h=8),
                    in1=sv.unsqueeze(3).broadcast_to([128, CHUNK, 8, 16]),
                    op=mybir.AluOpType.mult)
                nc.vector.tensor_copy(tv[:, :, HID:HID + 8], sv)
                m_t = mpool.tile([128, CHUNK * 128], F32, tag="mask")
                nc.vector.tensor_tensor(
                    out=m_t[:].rearrange("p (g d) -> p g d", g=CHUNK),
                    in0=rel_sb[:, c * CHUNK:(c + 1) * CHUNK].unsqueeze(2)
                        .broadcast_to([128, CHUNK, 128]),
                    in1=col[:].unsqueeze(1).broadcast_to([128, CHUNK, 128]),
                    op=mybir.AluOpType.is_equal)
                for s in range(CHUNK):
                    gs = c * CHUNK + s
                    w = slab_win[gs]
                    if w not in win_psum:
                        win_psum[w] = ps_w.tile([128, 136], F32, tag="psw",
                                                name=f"psw_{w}")
                    nc.tensor.matmul(
                        win_psum[w][:],
                        m_t[:, s * 128:(s + 1) * 128].bitcast(agg_dtype),
                        t_t[:, s * 136:(s + 1) * 136].bitcast(agg_dtype),
                        start=(gs == first_slab[w]), stop=(gs == last_slab[w]))
                    if gs == last_slab[w]:
                        evacuate(w)
            flush_ffn()
    nc.compile()
    return nc


# ----------------------------------------------------------------------------
# host orchestration
# ----------------------------------------------------------------------------

def kernel(**inputs):
    global LAST_STATS
    LAST_STATS = {}
    bench = int(os.environ.get("KERNEL_BENCH", "0"))

    fid = np.asarray(inputs["fid"]).astype(np.int64)
    sid = np.asarray(inputs["sid"]).astype(np.int64)
    uids = np.asarray(inputs["uids"]).astype(np.int64)
    iids = np.asarray(inputs["iids"]).astype(np.int64)
    src = np.asarray(inputs["src"]).astype(np.int64)
    dst = np.asarray(inputs["dst"]).astype(np.int64)

    tables = {
        "s": (np.asarray(inputs["sent_embed"], dtype=np.float32), sid, SENT_NUM),
        "f": (np.asarray(inputs["feature_embed"], dtype=np.float32), fid, FEAT_NUM),
        "u": (np.asarray(inputs["user_embed"], dtype=np.float32), uids, USER_NUM),
        "i": (np.asarray(inputs["item_embed"], dtype=np.float32), iids, ITEM_NUM),
    }
    Wt = {"s": np.asarray(inputs["Ws"], dtype=np.float32),
          "f": np.asarray(inputs["Wf"], dtype=np.float32),
          "u": np.asarray(inputs["Wu"], dtype=np.float32),
          "i": np.asarray(inputs["Wi"], dtype=np.float32)}

    # ---- node relabeling ----
    owner, local, counts = {}, {}, {}
    for t, (tab, ids, T_t) in tables.items():
        shard = T_t // NCORES
        own = ids // shard
        srt = np.argsort(own * np.int64(T_t + 1) + ids, kind="stable")
        cnt = np.bincount(own, minlength=NCORES)
        start = np.zeros(NCORES + 1, dtype=np.int64)
        start[1:] = np.cumsum(cnt)
        pos = np.empty(len(ids), dtype=np.int64)
        pos[srt] = np.arange(len(ids)) - start[own[srt]]
        owner[t], local[t], counts[t] = own, pos, cnt

    P_S = _ru(int(counts["s"].max()), NODE_ALIGN)
    P_F = _ru(int(counts["f"].max()), NODE_ALIGN)
    P_U = _ru(int(counts["u"].max()), NODE_ALIGN)
    P_I = _ru(int(counts["i"].max()), NODE_ALIGN)
    P_map = {"s": P_S, "f": P_F, "u": P_U, "i": P_I}
    CORE_ROWS = P_S + P_F + P_U + P_I
    GROWS = NCORES * CORE_ROWS
    t_off = {"s": 0, "f": P_S, "u": P_S + P_F, "i": P_S + P_F + P_U}

    loc_idx = {}
    for t, (tab, ids, T_t) in tables.items():
        shard = T_t // NCORES
        P_t = P_map[t]
        arrs = []
        for c in range(NCORES):
            li = np.zeros(P_t, dtype=np.int32)
            selc = np.where(owner[t] == c)[0]
            li[local[t][selc]] = (ids[selc] - c * shard).astype(np.int32)
            arrs.append(np.ascontiguousarray(li.reshape(P_t // 128, 128).T))
        loc_idx[t] = arrs

    g_row = np.empty(N_NODES, dtype=np.int64)
    node_base = {"f": 0, "s": N_FEAT, "u": N_FEAT + N_SENT,
                 "i": N_FEAT + N_SENT + N_USER}
    for t in ("f", "s", "u", "i"):
        nb = node_base[t]
        n_t = len(tables[t][1])
        g_row[nb:nb + n_t] = owner[t] * CORE_ROWS + t_off[t] + local[t]

    # ---- edges ----
    keep = (dst >= N_FEAT) & (dst < N_FEAT + N_SENT)
    e_src = src[keep]
    e_dst = dst[keep] - N_FEAT
    e_owner = owner["s"][e_dst]
    e_dlocal = local["s"][e_dst]
    e_srow = g_row[e_src]

    NW = P_S // 128
    # shared slab layout: per window, slab count = max over cores (>=1)
    cnt_w = np.zeros((NCORES, NW), dtype=np.int64)
    core_sorted = []
    for c in range(NCORES):
        sel = np.where(e_owner == c)[0]
        o = np.argsort(e_dlocal[sel], kind="stable")
        sel = sel[o]
        dl = e_dlocal[sel]
        wstart = np.searchsorted(dl, np.arange(0, P_S + 1, 128))
        for w in range(NW):
            cnt_w[c, w] = _ru(int(wstart[w + 1] - wstart[w]), 128) // 128
        core_sorted.append((sel, dl, wstart))
    req = np.maximum(cnt_w.max(axis=0), 1)
    SLABS = _ru(int(req.sum()), CHUNK)
    req[NW - 1] += (SLABS - int(req.sum()))
    slab_win = []
    for w in range(NW):
        slab_win.extend([w] * int(req[w]))

    core_edges = []
    for c in range(NCORES):
        sel, dl, wstart = core_sorted[c]
        sr = e_srow[sel]
        drow = c * CORE_ROWS + dl
        sw_l, dw_l, rw_l = [], [], []
        for w in range(NW):
            a, b = int(wstart[w]), int(wstart[w + 1])
            n = b - a
            npad = int(req[w]) * 128
            sw = np.zeros(npad, dtype=np.int32)
            dw = np.zeros(npad, dtype=np.int32)
            rw = np.full(npad, -1.0, dtype=np.float32)
            sw[:n] = sr[a:b]
            dw[:n] = drow[a:b]
            rw[:n] = (dl[a:b] - w * 128).astype(np.float32)
            sw_l.append(sw)
            dw_l.append(dw)
            rw_l.append(rw)
        sw = np.concatenate(sw_l)
        dw = np.concatenate(dw_l)
        rw = np.concatenate(rw_l)
        core_edges.append((
            np.ascontiguousarray(sw.reshape(SLABS, 128).T),
            np.ascontiguousarray(dw.reshape(SLABS, 128).T),
            np.ascontiguousarray(rw.reshape(SLABS, 128).T)))

    # ---- weights / constants ----
    Wg = np.asarray(inputs["Wg"], dtype=np.float32)
    attn_l = np.asarray(inputs["attn_l"], dtype=np.float32)
    attn_r = np.asarray(inputs["attn_r"], dtype=np.float32)
    alr = np.zeros((HID, 16), dtype=np.float32)
    for h in range(HEADS):
        alr[h * DH:(h + 1) * DH, h] = attn_l[h]
        alr[h * DH:(h + 1) * DH, 8 + h] = attn_r[h]
    ident_np = np.eye(128, dtype=np.float32)
    col_np = np.ascontiguousarray(
        np.tile(np.arange(128, dtype=np.float32)[None, :], (128, 1)))
    W1 = np.asarray(inputs["W1"], dtype=np.float32)
    b1 = np.asarray(inputs["b1"], dtype=np.float32)
    W2 = np.asarray(inputs["W2"], dtype=np.float32)
    b2 = np.asarray(inputs["b2"], dtype=np.float32)
    wh = np.asarray(inputs["wh"], dtype=np.float32)
    wh_b = np.asarray(inputs["wh_b"], dtype=np.float32)

    # ---- launch 1 ----
    shard_rows = {t: tables[t][2] // NCORES for t in tables}
    nc1 = _build_launch1(P_S, P_F, P_U, P_I, shard_rows)
    in_maps1 = []
    for c in range(NCORES):
        m = {"wgT": np.ascontiguousarray(Wg.T), "wg": Wg, "alr": alr,
             "ident": ident_np}
        for t in ("s", "f", "u", "i"):
            tab, ids, T_t = tables[t]
            shard = T_t // NCORES
            m[f"tbl_{t}"] = tab[c * shard:(c + 1) * shard]
            m[f"idx_{t}"] = loc_idx[t][c]
            m[f"wtT_{t}"] = np.ascontiguousarray(Wt[t].T)
        in_maps1.append(m)
    res1, t1 = _run_spmd(nc1, in_maps1, NCORES, bench=bench)
    H_full = np.ascontiguousarray(
        np.concatenate([res1[c]["H"] for c in range(NCORES)], axis=0))
    LAST_STATS["exec1_ns"] = int(t1 * 1e9) if t1 else None

    # ---- launch 2 ----
    nc2 = _build_launch2(GROWS, SLABS, slab_win, P_S)
    b1c = np.ascontiguousarray(b1.reshape(FFN // 128, 128).T)
    in_maps2 = []
    for c in range(NCORES):
        sw, dw, rw = core_edges[c]
        in_maps2.append({
            "H": H_full, "src": sw, "dst": dw, "rel": rw,
            "col": col_np, "ident": ident_np,
            "w1": W1, "b1c": b1c, "w2": W2,
            "b2c": np.ascontiguousarray(b2.reshape(128, 1)),
            "wh": wh, "whb": wh_b.reshape(1, 1).astype(np.float32)})
    res2, t2 = _run_spmd(nc2, in_maps2, NCORES, bench=bench)
    LAST_STATS["exec2_ns"] = int(t2 * 1e9) if t2 else None
    if os.environ.get("KERNEL_DEBUG"):
        LAST_STATS["H_full"] = H_full
        LAST_STATS["g_row"] = g_row
        LAST_STATS["owner_s"] = owner["s"]
        LAST_STATS["local_s"] = local["s"]
        LAST_STATS["CORE_ROWS"] = CORE_ROWS
        LAST_STATS["P_S"] = P_S
        LAST_STATS["core_edges"] = core_edges
        LAST_STATS["slab_win"] = slab_win
        LAST_STATS["res2"] = res2

    # ---- unpermute logits ----
    out = np.empty((N_SENT, 1), dtype=np.float32)
    for c in range(NCORES):
        lg = np.asarray(res2[c]["logits"]).reshape(-1)
        selc = np.where(owner["s"] == c)[0]
        out[selc, 0] = lg[local["s"][selc]]
    return out



# revision 11
# speedup vs baseline: 1.9155x; 1.9155x over previous
"""Trainium2 Bass kernel for heterogeneous GNN (GAT + FFN).

Single-launch design (the per-dispatch overhead through the PJRT path is
~70ms, so everything runs in ONE NEFF on NCORES cores):

  Phase A (replicated on every core): host pre-gathers embedding rows into
    node order, pre-transposes to [D, nodes] and casts bf16; the device
    streams xT tiles with plain sequential DMA, projects with fused weights
    W'' = W_t @ [Wg | Wg@Al | Wg@Ar] (bf16 matmuls), transposes back to
    row-major and writes H[node] = [h(128) | el(8) | er(8)] f32 rows into an
    internal DRAM table indexed by (padded) node id.

  Phase B (edges sharded by dst window across cores): edges with sentence
    dst only, sorted by dst, padded to 128-edge slabs.  Per slab: indirect
    gather of H[src]; attention logits z = el[src] + er[dst] where er[dst]
    comes from a per-window [128,8] tile via a transposed one-hot matmul
    (no per-edge dst gather); s = exp(leaky_relu(z)); segment softmax via
    one-hot mask matmuls accumulating [128-dst-window x (h.128 | den.8)] in
    PSUM; then normalize + ELU + FFN (residual) + logits.
"""

import os
import numpy as np

import concourse.bacc as bacc
import concourse.bass as bass
import concourse.mybir as mybir
import concourse.tile as tile


def _run_spmd(nc, in_maps, n_cores=8, bench=0):
    """Execute a compiled Bass program on n_cores via PJRT (axon).

    Pre-places inputs on device so repeated timed calls measure NEFF
    execution only.  Returns (results_per_core, best_exec_seconds or None).
    """
    import time as _time
    import jax
    from jax.sharding import Mesh, PartitionSpec, NamedSharding
    from jax.experimental.shard_map import shard_map
    from concourse import bass2jax as b2j
    from concourse import mybir as mb

    b2j.install_neuronx_cc_hook()
    part_name = nc.partition_id_tensor.name if nc.partition_id_tensor else None
    in_names, out_names, out_avals, zero_outs = [], [], [], []
    for alloc in nc.m.functions[0].allocations:
        if not isinstance(alloc, mb.MemoryLocationSet):
            continue
        name = alloc.memorylocations[0].name
        if alloc.kind == "ExternalInput":
            if name != part_name:
                in_names.append(name)
        elif alloc.kind == "ExternalOutput":
            out_names.append(name)
            shape = tuple(alloc.tensor_shape)
            dtype = mb.dt.np(alloc.dtype)
            out_avals.append(jax.core.ShapedArray(shape, dtype))
            zero_outs.append(np.zeros(shape, dtype))
    n_params = len(in_names)
    n_outs = len(out_avals)
    all_names = in_names + out_names
    if part_name is not None:
        all_names = all_names + [part_name]

    def _body(*args):
        operands = list(args)
        if part_name is not None:
            operands.append(b2j.partition_id_tensor())
        outs = b2j._bass_exec_p.bind(
            *operands,
            out_avals=tuple(out_avals),
            in_names=tuple(all_names),
            out_names=tuple(out_names),
            lowering_input_output_aliases=(),
            sim_require_finite=True,
            sim_require_nnan=True,
            nc=nc,
        )
        return tuple(outs)

    devices = jax.devices()[:n_cores]
    mesh = Mesh(np.asarray(devices), ("core",))
    donate = tuple(range(n_params, n_params + n_outs))
    sharded = jax.jit(
        shard_map(_body, mesh=mesh,
                  in_specs=(PartitionSpec("core"),) * (n_params + n_outs),
                  out_specs=(PartitionSpec("core"),) * n_outs,
                  check_rep=False),
        donate_argnums=donate, keep_unused=True)
    spec = NamedSharding(mesh, PartitionSpec("core"))
    concat_in = [
        jax.device_put(
            np.concatenate([np.asarray(in_maps[c][nm]) for c in range(n_cores)],
                           axis=0), spec)
        for nm in in_names
    ]
    def _zeros():
        return [jax.device_put(
                    np.zeros((n_cores * z.shape[0], *z.shape[1:]), z.dtype),
                    spec)
                for z in zero_outs]

    out_arrs = sharded(*concat_in, *_zeros())
    jax.block_until_ready(out_arrs)
    results = [
        {nm: np.asarray(out_arrs[i]).reshape(n_cores, *out_avals[i].shape)[c]
         for i, nm in enumerate(out_names)}
        for c in range(n_cores)
    ]
    best = None
    for _ in range(bench):
        zz = _zeros()
        jax.block_until_ready(zz)
        t0 = _time.perf_counter()
        oo = sharded(*concat_in, *zz)
        jax.block_until_ready(oo)
        dt_s = _time.perf_counter() - t0
        best = dt_s if best is None or dt_s < best else best
    return results, best


N_FEAT, N_SENT, N_USER, N_ITEM = 60000, 100000, 5000, 5000
FEAT_NUM, SENT_NUM, USER_NUM, ITEM_NUM = 60000, 200000, 100000, 50000
HEADS, DH, HID = 8, 16, 128
FFN = 512
ROW = 144                     # h(128) | el(8) | er(8)
NCORES = 4
SUP = 2048                    # nodes per phase-A superblock
CHUNK = 16                    # slabs per phase-2 chunk (2048 edges)

F32 = mybir.dt.float32
F32R = mybir.dt.float32r
BF16 = mybir.dt.bfloat16
I32 = mybir.dt.int32

# per-type padded node counts / H-row bases (feat | sent | user | item)
P_FEAT = 61440
P_SENT = 100352
P_USER = 6144
P_ITEM = 6144
FB, SB = 0, P_FEAT
UB = SB + P_SENT
IB = UB + P_USER
R_TOT = IB + P_ITEM

NW_TOT = (N_SENT + 127) // 128           # 782 real dst windows
NW_CORE = (NW_TOT + NCORES - 1) // NCORES  # local windows per core
P_S_CORE = NW_CORE * 128                 # logits per core

LAST_STATS = {}


def _ru(x, m):
    return ((x + m - 1) // m) * m


def _build(n_slabs, slab_win, xdims):
    """One NEFF: phase A (replicated projection) + phase B (edge agg)."""
    nc = bacc.Bacc("TRN2", target_bir_lowering=False, debug=False,
                   enable_asserts=False)

    types = [("s", xdims["s"], P_SENT, SB),
             ("f", xdims["f"], P_FEAT, FB),
             ("u", xdims["u"], P_USER, UB),
             ("i", xdims["i"], P_ITEM, IB)]

    xs_d, wp_d = {}, {}
    for t, D, P_t, base in types:
        xs_d[t] = nc.dram_tensor(f"xs_{t}", [D, P_t], BF16,
                                 kind="ExternalInput").ap()
        wp_d[t] = nc.dram_tensor(f"wp_{t}", [D, ROW], BF16,
                                 kind="ExternalInput").ap()
    identb_d = nc.dram_tensor("identb", [128, 128], BF16, kind="ExternalInput").ap()
    identf_d = nc.dram_tensor("identf", [128, 128], F32, kind="ExternalInput").ap()
    col_d = nc.dram_tensor("col", [128, 128], F32, kind="ExternalInput").ap()
    rowv_d = nc.dram_tensor("rowv", [128, 1], F32, kind="ExternalInput").ap()
    ones_d = nc.dram_tensor("ones1", [1, 128], F32, kind="ExternalInput").ap()
    src_d = nc.dram_tensor("src", [128, n_slabs], I32, kind="ExternalInput").ap()
    rel_d = nc.dram_tensor("rel", [128, n_slabs], F32, kind="ExternalInput").ap()
    relT_d = nc.dram_tensor("relT", [1, n_slabs * 128], F32, kind="ExternalInput").ap()
    w1_d = nc.dram_tensor("w1", [HID, FFN], BF16, kind="ExternalInput").ap()
    b1_d = nc.dram_tensor("b1c", [128, FFN // 128], F32, kind="ExternalInput").ap()
    w2_d = nc.dram_tensor("w2", [FFN, HID], BF16, kind="ExternalInput").ap()
    b2_d = nc.dram_tensor("b2c", [128, 1], F32, kind="ExternalInput").ap()
    wh_d = nc.dram_tensor("wh", [HID, 1], BF16, kind="ExternalInput").ap()
    whb_d = nc.dram_tensor("whb", [1, 1], F32, kind="ExternalInput").ap()
    lg_d = nc.dram_tensor("logits", [1, P_S_CORE], F32, kind="ExternalOutput").ap()

    H_d = nc.dram_tensor("Htab", [R_TOT, ROW], BF16, kind="Internal").ap()

    first_slab, last_slab = {}, {}
    for s, w in enumerate(slab_win):
        first_slab.setdefault(w, s)
        last_slab[w] = s
    n_chunks = n_slabs // CHUNK

    with tile.TileContext(nc) as tc:
        # ------------------------------------------------------------------
        # phase A: stream xT, project, write H rows
        # ------------------------------------------------------------------
        with (
            tc.tile_pool(name="acst", bufs=1) as acpool,
            tc.tile_pool(name="xt", bufs=2) as xtpool,
            tc.tile_pool(name="ht", bufs=2) as htpool,
            tc.tile_pool(name="hrow", bufs=3) as hrowpool,
            tc.tile_pool(name="ps_h", bufs=2, space="PSUM") as ps_h,
            tc.tile_pool(name="ps_e", bufs=2, space="PSUM") as ps_e,
            tc.tile_pool(name="ps_o", bufs=2, space="PSUM") as ps_o,
        ):
            identb = acpool.tile([128, 128], BF16)
            nc.sync.dma_start(identb[:], identb_d[:])
            wp_sb = {}
            for t, D, P_t, base in types:
                wp_sb[t] = acpool.tile([128, (D // 128) * ROW], BF16,
                                       tag=f"wp{t}", name=f"wp_sb_{t}")
                for ci in range(D // 128):
                    nc.sync.dma_start(
                        wp_sb[t][:, ci * ROW:(ci + 1) * ROW],
                        wp_d[t][ci * 128:(ci + 1) * 128, :])

            for t, D, P_t, base in types:
                nck = D // 128
                for sbk in range(P_t // SUP):
                    xt = xtpool.tile([128, (SUP // 2048) * 2048 * 6], BF16,
                                     tag="xt", name=f"xt_{t}_{sbk}")
                    for ci in range(nck):
                        nc.sync.dma_start(
                            xt[:, ci * SUP:(ci + 1) * SUP],
                            xs_d[t][ci * 128:(ci + 1) * 128,
                                    sbk * SUP:(sbk + 1) * SUP])
                    for q in range(SUP // 512):
                        psh = ps_h.tile([128, 512], F32, tag="psh")
                        pse = ps_e.tile([16, 512], F32, tag="pse")
                        for ci in range(nck):
                            rhs = xt[:, ci * SUP + q * 512: ci * SUP + q * 512 + 512]
                            nc.tensor.matmul(psh[:],
                                             wp_sb[t][:, ci * ROW:ci * ROW + HID],
                                             rhs, start=(ci == 0),
                                             stop=(ci == nck - 1))
                            nc.tensor.matmul(pse[0:16, :],
                                             wp_sb[t][:, ci * ROW + HID:(ci + 1) * ROW],
                                             rhs, start=(ci == 0),
                                             stop=(ci == nck - 1))
                        ht = htpool.tile([128, 512], BF16, tag="ht")
                        et = htpool.tile([16, 512], BF16, tag="et")
                        nc.scalar.activation(ht[:], psh[:],
                                             mybir.ActivationFunctionType.Identity)
                        nc.scalar.activation(et[:], pse[0:16, :],
                                             mybir.ActivationFunctionType.Identity)
                        for g in range(4):
                            pso = ps_o.tile([128, ROW], BF16, tag="pso")
                            nc.tensor.transpose(
                                out=pso[:, 0:HID],
                                in_=ht[:, g * 128:(g + 1) * 128],
                                identity=identb[:])
                            nc.tensor.transpose(
                                out=pso[:, HID:ROW],
                                in_=et[0:16, g * 128:(g + 1) * 128],
                                identity=identb[0:16, 0:16])
                            hrow = hrowpool.tile([128, ROW], BF16, tag="hrow")
                            nc.vector.tensor_copy(hrow[:], pso[:])
                            r0 = base + sbk * SUP + q * 512 + g * 128
                            nc.sync.dma_start(H_d[r0:r0 + 128, :], hrow[:])

        # ------------------------------------------------------------------
        # phase B: edge aggregation + FFN + logits
        # ------------------------------------------------------------------
        with (
            tc.tile_pool(name="const", bufs=1) as cpool,
            tc.tile_pool(name="g", bufs=2) as gpool,
            tc.tile_pool(name="t", bufs=2) as tpool,
            tc.tile_pool(name="m", bufs=2) as mpool,
            tc.tile_pool(name="mt", bufs=2) as mtpool,
            tc.tile_pool(name="sm", bufs=3) as smpool,
            tc.tile_pool(name="er", bufs=3) as erpool,
            tc.tile_pool(name="ev", bufs=3) as evpool,
            tc.tile_pool(name="xf", bufs=2) as xfpool,
            tc.tile_pool(name="y", bufs=2) as ypool,
            tc.tile_pool(name="ps_w", bufs=2, space="PSUM") as ps_w,
            tc.tile_pool(name="ps_tp", bufs=1, space="PSUM") as ps_tp,
            tc.tile_pool(name="ps_bc", bufs=1, space="PSUM") as ps_bc,
            tc.tile_pool(name="ps_er", bufs=1, space="PSUM") as ps_er,
            tc.tile_pool(name="ps_y", bufs=1, space="PSUM") as ps_y,
            tc.tile_pool(name="ps_z", bufs=1, space="PSUM") as ps_z,
            tc.tile_pool(name="ps_l", bufs=1, space="PSUM") as ps_l,
        ):
            col = cpool.tile([128, 128], F32)
            rowv = cpool.tile([128, 1], F32)
            ones1 = cpool.tile([1, 128], F32)
            identf = cpool.tile([128, 128], F32)
            w1_t = cpool.tile([HID, FFN], BF16)
            b1_t = cpool.tile([128, FFN // 128], F32)
            w2_t = cpool.tile([128, FFN], BF16)  # block j = W2[j*128:(j+1)*128,:]
            b2_t = cpool.tile([128, 1], F32)
            wh_t = cpool.tile([HID, 1], BF16)
            whb_t = cpool.tile([1, 1], F32)
            src_sb = cpool.tile([128, n_slabs], I32)
            rel_sb = cpool.tile([128, n_slabs], F32)
            nc.sync.dma_start(col[:], col_d[:])
            nc.sync.dma_start(rowv[:], rowv_d[:])
            nc.sync.dma_start(ones1[:], ones_d[:])
            nc.sync.dma_start(identf[:], identf_d[:])
            nc.sync.dma_start(w1_t[:], w1_d[:])
            nc.sync.dma_start(b1_t[:], b1_d[:])
            for j in range(FFN // 128):
                nc.sync.dma_start(w2_t[:, j * 128:(j + 1) * 128],
                                  w2_d[j * 128:(j + 1) * 128, :])
            nc.sync.dma_start(b2_t[:], b2_d[:])
            nc.sync.dma_start(wh_t[:], wh_d[:])
            nc.sync.dma_start(whb_t[:], whb_d[:])
            nc.sync.dma_start(src_sb[:], src_d[:])
            nc.sync.dma_start(rel_sb[:], rel_d[:])

            win_psum = {}
            win_er = {}
            xf_state = {"tile": None, "count": 0, "base": 0}

            def flush_ffn():
                nbat = xf_state["count"]
                if nbat == 0:
                    return
                xf = xf_state["tile"]
                nb = nbat * 128
                xfr = ypool.tile([128, 512], BF16, tag="xfr")
                nc.vector.tensor_copy(xfr[:, 0:nb], xf[:, 0:nb])
                yts = []
                for j in range(FFN // 128):
                    psy = ps_y.tile([128, 512], F32, tag="psy")
                    nc.tensor.matmul(
                        psy[:, 0:nb],
                        w1_t[:, j * 128:(j + 1) * 128],
                        xfr[:, 0:nb],
                        start=True, stop=True)
                    y_t = ypool.tile([128, 512], BF16, tag="y")
                    nc.scalar.activation(y_t[:, 0:nb], psy[:, 0:nb],
                                         mybir.ActivationFunctionType.Relu,
                                         bias=b1_t[:, j:j + 1])
                    yts.append(y_t)
                psz = ps_z.tile([128, 512], F32, tag="psz")
                for j in range(FFN // 128):
                    nc.tensor.matmul(
                        psz[:, 0:nb],
                        w2_t[:, j * 128:(j + 1) * 128],
                        yts[j][:, 0:nb],
                        start=(j == 0), stop=(j == FFN // 128 - 1))
                z_t = evpool.tile([128, 512], F32, tag="z")
                nc.scalar.activation(z_t[:, 0:nb], psz[:, 0:nb],
                                     mybir.ActivationFunctionType.Identity,
                                     bias=b2_t[:, 0:1])
                nc.vector.tensor_tensor(out=z_t[:, 0:nb], in0=z_t[:, 0:nb],
                                        in1=xf[:, 0:nb], op=mybir.AluOpType.add)
                zb_t = evpool.tile([128, 512], BF16, tag="zb")
                nc.vector.tensor_copy(zb_t[:, 0:nb], z_t[:, 0:nb])
                psl = ps_l.tile([1, 512], F32, tag="psl")
                nc.tensor.matmul(psl[0:1, 0:nb], wh_t[:], zb_t[:, 0:nb],
                                 start=True, stop=True)
                lg_t = evpool.tile([1, 512], F32, tag="lg")
                nc.scalar.activation(lg_t[0:1, 0:nb], psl[0:1, 0:nb],
                                     mybir.ActivationFunctionType.Identity,
                                     bias=whb_t[0:1, 0:1])
                b0 = xf_state["base"] * 128
                nc.sync.dma_start(lg_d[0:1, b0:b0 + nb], lg_t[0:1, 0:nb])
                xf_state["tile"] = None
                xf_state["count"] = 0

            def evacuate(w):
                psw = win_psum.pop(w)
                win_er.pop(w)
                den = evpool.tile([128, 8], F32, tag="den")
                nc.vector.tensor_scalar(out=den[:], in0=psw[:, HID:HID + 8],
                                        scalar1=1e-9, scalar2=None,
                                        op0=mybir.AluOpType.add)
                rcp = evpool.tile([128, 8], F32, tag="rcp")
                nc.vector.reciprocal(rcp[:], den[:])
                ot = evpool.tile([128, 128], F32, tag="ot")
                nc.vector.tensor_tensor(
                    out=ot[:].rearrange("p (h r) -> p h r", h=8),
                    in0=psw[:, 0:HID].rearrange("p (h r) -> p h r", h=8),
                    in1=rcp[:].unsqueeze(2).broadcast_to([128, 8, 16]),
                    op=mybir.AluOpType.mult)
                neg = evpool.tile([128, 128], F32, tag="neg")
                nc.vector.tensor_scalar(out=neg[:], in0=ot[:], scalar1=0.0,
                                        scalar2=None, op0=mybir.AluOpType.min)
                emn = evpool.tile([128, 128], F32, tag="emn")
                nc.scalar.activation(emn[:], neg[:],
                                     mybir.ActivationFunctionType.Exp)
                pos = evpool.tile([128, 128], F32, tag="pos")
                nc.vector.tensor_scalar(out=pos[:], in0=ot[:], scalar1=0.0,
                                        scalar2=None, op0=mybir.AluOpType.max)
                nc.vector.tensor_scalar(out=emn[:], in0=emn[:], scalar1=-1.0,
                                        scalar2=None, op0=mybir.AluOpType.add)
                elu = evpool.tile([128, 128], F32, tag="elu")
                nc.vector.tensor_tensor(out=elu[:], in0=pos[:], in1=emn[:],
                                        op=mybir.AluOpType.add)
                pst = ps_tp.tile([128, 128], F32, tag="pstp")
                nc.tensor.transpose(out=pst[:], in_=elu[:], identity=identf[:])
                if xf_state["tile"] is None:
                    xf_state["tile"] = xfpool.tile([128, 512], F32, tag="xf",
                                                   name=f"xf_{w}")
                    xf_state["base"] = w
                k = xf_state["count"]
                nc.vector.tensor_copy(
                    xf_state["tile"][:, k * 128:(k + 1) * 128], pst[:])
                xf_state["count"] = k + 1
                if xf_state["count"] == 4:
                    flush_ffn()

            for c in range(n_chunks):
                g_t = gpool.tile([128, CHUNK * ROW], BF16, tag="g")
                for g in range(CHUNK):
                    sc = c * CHUNK + g
                    nc.gpsimd.indirect_dma_start(
                        out=g_t[:, g * ROW:(g + 1) * ROW], out_offset=None,
                        in_=H_d[:],
                        in_offset=bass.IndirectOffsetOnAxis(
                            ap=src_sb[:, sc:sc + 1], axis=0))
                # transposed one-hot masks for the whole chunk:
                # maskT[d, e] = (d == rel[e])
                relT = smpool.tile([1, CHUNK * 128], F32, tag="rT")
                nc.sync.dma_start(
                    relT[:], relT_d[0:1, c * CHUNK * 128:(c + 1) * CHUNK * 128])
                mT = mtpool.tile([128, CHUNK * 128], BF16, tag="mT")
                for q in range(CHUNK * 128 // 512):
                    psb = ps_bc.tile([128, 512], F32, tag="psb")
                    nc.tensor.matmul(psb[:], ones1[0:1, :],
                                     relT[0:1, q * 512:(q + 1) * 512],
                                     start=True, stop=True)
                    nc.vector.tensor_tensor(
                        out=mT[:, q * 512:(q + 1) * 512], in0=psb[:],
                        in1=rowv[:, 0:1].broadcast_to([128, 512]),
                        op=mybir.AluOpType.is_equal)
                # per-edge er via maskT @ er_win, then z = el + er
                z_t = smpool.tile([128, CHUNK * 8], F32, tag="z8")
                gv = g_t[:].rearrange("p (g r) -> p g r", g=CHUNK)
                elc = smpool.tile([128, CHUNK * 8], F32, tag="elc")
                nc.vector.tensor_copy(
                    elc[:].rearrange("p (g h) -> p g h", g=CHUNK),
                    gv[:, :, HID:HID + 8])
                elv = elc[:].rearrange("p (g h) -> p g h", g=CHUNK)
                for g in range(CHUNK):
                    sc = c * CHUNK + g
                    w = slab_win[sc]
                    if w not in win_er:
                        ert = erpool.tile([128, 8], BF16, tag="er",
                                          name=f"er_{w}")
                        gw0 = SB + (w * 128)
                        nc.sync.dma_start(ert[:],
                                          H_d[gw0:gw0 + 128, HID + 8:ROW])
                        win_er[w] = ert
                    pser = ps_er.tile([128, 8], F32, tag="pser")
                    nc.tensor.matmul(pser[:],
                                     mT[:, g * 128:(g + 1) * 128],
                                     win_er[w][:], start=True, stop=True)
                    nc.vector.tensor_tensor(
                        out=z_t[:, g * 8:(g + 1) * 8],
                        in0=elv[:, g, :],
                        in1=pser[:], op=mybir.AluOpType.add)
                zz_t = smpool.tile([128, CHUNK * 8], F32, tag="zz8")
                nc.scalar.mul(zz_t[:], z_t[:], 0.2)
                nc.vector.tensor_tensor(out=z_t[:], in0=z_t[:], in1=zz_t[:],
                                        op=mybir.AluOpType.max)
                s_t = smpool.tile([128, CHUNK * 8], F32, tag="s8")
                nc.scalar.activation(s_t[:], z_t[:],
                                     mybir.ActivationFunctionType.Exp)
                sb_t = smpool.tile([128, CHUNK * 8], BF16, tag="sb8")
                nc.vector.tensor_copy(sb_t[:], s_t[:])
                sv = sb_t[:].rearrange("p (g h) -> p g h", g=CHUNK)
                t_t = tpool.tile([128, CHUNK * 136], BF16, tag="t")
                tv = t_t[:].rearrange("p (g c) -> p g c", g=CHUNK)
                nc.vector.tensor_tensor(
                    out=tv[:, :, 0:HID].rearrange("p g (h r) -> p g h r", h=8),
                    in0=gv[:, :, 0:HID].rearrange("p g (h r) -> p g h r", h=8),
                    in1=sv.unsqueeze(3).broadcast_to([128, CHUNK, 8, 16]),
                    op=mybir.AluOpType.mult)
                nc.vector.tensor_copy(tv[:, :, HID:HID + 8], sv)
                m_t = mpool.tile([128, CHUNK * 128], BF16, tag="mask")
                nc.vector.tensor_tensor(
                    out=m_t[:].rearrange("p (g d) -> p g d", g=CHUNK),
                    in0=rel_sb[:, c * CHUNK:(c + 1) * CHUNK].unsqueeze(2)
                        .broadcast_to([128, CHUNK, 128]),
                    in1=col[:].unsqueeze(1).broadcast_to([128, CHUNK, 128]),
                    op=mybir.AluOpType.is_equal)
                for s in range(CHUNK):
                    gs = c * CHUNK + s
                    w = slab_win[gs]
                    if w not in win_psum:
                        win_psum[w] = ps_w.tile([128, 136], F32, tag="psw",
                                                name=f"psw_{w}")
                    nc.tensor.matmul(
                        win_psum[w][:],
                        m_t[:, s * 128:(s + 1) * 128],
                        t_t[:, s * 136:(s + 1) * 136],
                        start=(gs == first_slab[w]), stop=(gs == last_slab[w]))
                    if gs == last_slab[w]:
                        evacuate(w)
            flush_ffn()
    nc.compile()
    return nc


# ----------------------------------------------------------------------------
# host orchestration
# ----------------------------------------------------------------------------

def kernel(**inputs):
    global LAST_STATS
    LAST_STATS = {}
    import ml_dtypes
    bf16 = ml_dtypes.bfloat16
    bench = int(os.environ.get("KERNEL_BENCH", "0"))

    fid = np.asarray(inputs["fid"]).astype(np.int64)
    sid = np.asarray(inputs["sid"]).astype(np.int64)
    uids = np.asarray(inputs["uids"]).astype(np.int64)
    iids = np.asarray(inputs["iids"]).astype(np.int64)
    src = np.asarray(inputs["src"]).astype(np.int64)
    dst = np.asarray(inputs["dst"]).astype(np.int64)

    Wg = np.asarray(inputs["Wg"], dtype=np.float32)
    attn_l = np.asarray(inputs["attn_l"], dtype=np.float32)
    attn_r = np.asarray(inputs["attn_r"], dtype=np.float32)
    AL = np.zeros((HID, HEADS), dtype=np.float32)
    AR = np.zeros((HID, HEADS), dtype=np.float32)
    for h in range(HEADS):
        AL[h * DH:(h + 1) * DH, h] = attn_l[h]
        AR[h * DH:(h + 1) * DH, h] = attn_r[h]
    Wg144 = np.concatenate([Wg, Wg @ AL, Wg @ AR], axis=1)  # [128,144]

    # per-type node-ordered, padded, transposed bf16 activations + fused W''
    specs = [
        ("s", np.asarray(inputs["sent_embed"], dtype=np.float32), sid,
         np.asarray(inputs["Ws"], dtype=np.float32), P_SENT, 768),
        ("f", np.asarray(inputs["feature_embed"], dtype=np.float32), fid,
         np.asarray(inputs["Wf"], dtype=np.float32), P_FEAT, 384),
        ("u", np.asarray(inputs["user_embed"], dtype=np.float32), uids,
         np.asarray(inputs["Wu"], dtype=np.float32), P_USER, 128),
        ("i", np.asarray(inputs["item_embed"], dtype=np.float32), iids,
         np.asarray(inputs["Wi"], dtype=np.float32), P_ITEM, 128),
    ]
    xs_np, wp_np, xdims = {}, {}, {}
    for t, tab, ids, Wt, P_t, D_pad in specs:
        D = tab.shape[1]
        xT = np.zeros((D_pad, P_t), dtype=bf16)
        xT[:D, :len(ids)] = tab[ids].T.astype(bf16)
        xs_np[t] = xT
        wp = np.zeros((D_pad, ROW), dtype=np.float32)
        wp[:D, :] = Wt @ Wg144
        wp_np[t] = wp.astype(bf16)
        xdims[t] = D_pad

    # H-row id for each global node.  The sentence block of the H table is
    # ROTATED per core by -c*P_S_CORE (via rotated xs_s input) so that each
    # core's own dst windows sit at the fixed rows SB + w*128 the shared
    # NEFF loads er from.
    row_of = np.empty(N_FEAT + N_SENT + N_USER + N_ITEM, dtype=np.int64)
    row_of[:N_FEAT] = FB + np.arange(N_FEAT)
    row_of[N_FEAT:N_FEAT + N_SENT] = SB + np.arange(N_SENT)
    row_of[N_FEAT + N_SENT:N_FEAT + N_SENT + N_USER] = UB + np.arange(N_USER)
    row_of[N_FEAT + N_SENT + N_USER:] = IB + np.arange(N_ITEM)

    # ---- edges (sentence dst only), sharded by dst window ----
    keep = (dst >= N_FEAT) & (dst < N_FEAT + N_SENT)
    e_src = src[keep]
    e_d = dst[keep] - N_FEAT
    e_srow = row_of[e_src]
    e_sent = (e_src >= N_FEAT) & (e_src < N_FEAT + N_SENT)
    e_sl = e_src[e_sent] - N_FEAT
    e_w = e_d // 128
    e_core = np.minimum(e_w // NW_CORE, NCORES - 1)

    core_sorted = []
    cnt_w = np.zeros((NCORES, NW_CORE), dtype=np.int64)
    for c in range(NCORES):
        sel = np.where(e_core == c)[0]
        o = np.argsort(e_d[sel], kind="stable")
        sel = sel[o]
        dl = e_d[sel] - c * NW_CORE * 128
        wstart = np.searchsorted(dl, np.arange(0, NW_CORE * 128 + 1, 128))
        for w in range(NW_CORE):
            cnt_w[c, w] = _ru(max(int(wstart[w + 1] - wstart[w]), 1), 128) // 128
        core_sorted.append((sel, dl, wstart))
    req = cnt_w.max(axis=0)
    SLABS = _ru(int(req.sum()), CHUNK)
    req[NW_CORE - 1] += SLABS - int(req.sum())
    slab_win = []
    for w in range(NW_CORE):
        slab_win.extend([w] * int(req[w]))

    core_edges = []
    for c in range(NCORES):
        sel, dl, wstart = core_sorted[c]
        srow_c = e_srow.copy()
        srow_c[e_sent] = SB + (e_sl - c * P_S_CORE) % P_SENT
        sr = srow_c[sel]
        sw_l, rw_l = [], []
        for w in range(NW_CORE):
            a, b = int(wstart[w]), int(wstart[w + 1])
            n = b - a
            npad = int(req[w]) * 128
            sw = np.zeros(npad, dtype=np.int32)
            rw = np.full(npad, -1.0, dtype=np.float32)
            sw[:n] = sr[a:b]
            rw[:n] = (dl[a:b] - w * 128).astype(np.float32)
            sw_l.append(sw)
            rw_l.append(rw)
        sw = np.concatenate(sw_l)
        rw = np.concatenate(rw_l)
        core_edges.append((
            np.ascontiguousarray(sw.reshape(SLABS, 128).T),
            np.ascontiguousarray(rw.reshape(SLABS, 128).T),
            rw.reshape(1, SLABS * 128).astype(np.float32)))

    # ---- constants / weights ----
    identb_np = np.eye(128, dtype=bf16)
    identf_np = np.eye(128, dtype=np.float32)
    col_np = np.ascontiguousarray(
        np.tile(np.arange(128, dtype=np.float32)[None, :], (128, 1)))
    rowv_np = np.arange(128, dtype=np.float32).reshape(128, 1)
    ones_np = np.ones((1, 128), dtype=np.float32)
    W1 = np.asarray(inputs["W1"], dtype=np.float32)
    b1 = np.asarray(inputs["b1"], dtype=np.float32)
    W2 = np.asarray(inputs["W2"], dtype=np.float32)
    b2 = np.asarray(inputs["b2"], dtype=np.float32)
    wh = np.asarray(inputs["wh"], dtype=np.float32)
    wh_b = np.asarray(inputs["wh_b"], dtype=np.float32)
    b1c = np.ascontiguousarray(b1.reshape(FFN // 128, 128).T)

    nc = _build(SLABS, slab_win, xdims)
    in_maps = []
    for c in range(NCORES):
        sw, rw, rwT = core_edges[c]
        m = {"identb": identb_np, "identf": identf_np, "col": col_np,
             "rowv": rowv_np, "ones1": ones_np,
             "src": sw, "rel": rw, "relT": rwT,
             "w1": W1.astype(bf16), "b1c": b1c, "w2": W2.astype(bf16),
             "b2c": np.ascontiguousarray(b2.reshape(128, 1)),
             "wh": wh.astype(bf16), "whb": wh_b.reshape(1, 1).astype(np.float32)}
        for t in ("s", "f", "u", "i"):
            if t == "s" and c > 0:
                m[f"xs_{t}"] = np.roll(xs_np[t], -c * P_S_CORE, axis=1)
            else:
                m[f"xs_{t}"] = xs_np[t]
            m[f"wp_{t}"] = wp_np[t]
        in_maps.append(m)
    res, t1 = _run_spmd(nc, in_maps, NCORES, bench=bench)
    LAST_STATS["exec1_ns"] = int(t1 * 1e9) if t1 else None

    out = np.concatenate([np.asarray(res[c]["logits"]).reshape(-1)
                          for c in range(NCORES)])[:N_SENT]
    return np.ascontiguousarray(out.reshape(N_SENT, 1)).astype(np.float32)
